# revision 7
# baseline (speedup 1.0000x reference)
"""Bahdanau attention mechanism on 8 Trainium2 NeuronCores.

Problem (full shapes): B=32, T=4096, QD=VD=AD=256
    keys = values @ Wv.T + bv            [B, T, AD]
    q    = query @ Wq.T + bq             [B, AD]
    e    = tanh(q[:,None,:] + keys) @ wv_score + bv_score   [B, T]
    a    = softmax(e, axis=-1)           [B, T]
    c    = einsum('bt,btv->bv', a, values)

Sharding: data-parallel over batch. Each of the 8 cores gets 4 batches;
all weights replicated. No collectives.

Per-core dataflow (all matmul inputs bf16, fp32 accumulation):
  - values are DMA'd HBM->SBUF with an f32->bf16 cast (SWDGE).
  - PE transposes 128x128 blocks of values to get v^T for the keys matmul
    (bf16 transposes write bf16 PSUM, evacuated by DVE at 2x rate).
  - keys^T[a, t] accumulates in PSUM; ACT applies tanh with the per-partition
    bias q[a]+bv[a] and writes bf16 tanh^T tiles.
  - e is reduced over a with a matmul against a one-hot-shifted "band" of
    wv_score so that each 512-wide t-slice of e lands in its own PSUM row
    (rows 32b+s, all contiguous) - avoids M=1 output-base restrictions.
  - exp (no max subtraction: |e| <= ~4 so exp is safe in fp32) with
    accum_out produces the softmax denominator for free.
  - a is normalized on DVE, streamed out with a bf16->f32 cast DMA, and
    transposed on PE ([32,128] blocks) to feed the c matmul.
  - c[b] = a_norm @ values accumulates in PSUM row 32b and is copied out.

bv_score is mathematically irrelevant (softmax shift invariance) and the
outputs (c, a) do not depend on it, so the kernel ignores it.
"""

import numpy as np

import concourse.bacc as bacc
import concourse.mybir as mybir
import concourse.tile as tile
from concourse import masks
from concourse.bass_utils import run_bass_kernel_spmd

N_CORES = 8
B = 32
B_LOC = B // N_CORES  # 4
T = 4096
D = 256  # QD == VD == AD

F32 = mybir.dt.float32
BF16 = mybir.dt.bfloat16
AF = mybir.ActivationFunctionType

_NC_CACHE = None


def build_nc():
    """Build (and cache) the per-core Bass program."""
    global _NC_CACHE
    if _NC_CACHE is not None:
        return _NC_CACHE

    nc = bacc.Bacc(
        "TRN2", target_bir_lowering=False, debug=False, num_devices=N_CORES
    )

    values = nc.dram_tensor("values", [B_LOC, T, D], F32, kind="ExternalInput").ap()
    query = nc.dram_tensor("query", [B_LOC, D], F32, kind="ExternalInput").ap()
    Wq = nc.dram_tensor("Wq", [D, D], F32, kind="ExternalInput").ap()
    bq = nc.dram_tensor("bq", [D], F32, kind="ExternalInput").ap()
    Wv = nc.dram_tensor("Wv", [D, D], F32, kind="ExternalInput").ap()
    bv = nc.dram_tensor("bv", [D], F32, kind="ExternalInput").ap()
    wv_score = nc.dram_tensor("wv_score", [D], F32, kind="ExternalInput").ap()
    c_out = nc.dram_tensor("c_out", [B_LOC, D], F32, kind="ExternalOutput").ap()
    a_out = nc.dram_tensor("a_out", [B_LOC, T], F32, kind="ExternalOutput").ap()

    with tile.TileContext(nc) as tc:
        with (
            tc.tile_pool(name="const", bufs=1) as const_pool,
            tc.tile_pool(name="vnat", bufs=2) as vnat_pool,
            tc.tile_pool(name="vtsb", bufs=4) as vtsb_pool,
            tc.tile_pool(name="tanh", bufs=3) as tanh_pool,
            tc.tile_pool(name="atsb", bufs=2) as atsb_pool,
            tc.tile_pool(name="ps_vt", bufs=2, space="PSUM") as ps_vt,  # 2 banks
            tc.tile_pool(name="ps_keys", bufs=2, space="PSUM") as ps_keys,  # 4 banks
            tc.tile_pool(name="ps_e", bufs=1, space="PSUM") as ps_e,  # 1 bank
            tc.tile_pool(name="ps_c", bufs=1, space="PSUM") as ps_c,  # 1 bank
        ):
            # ---------------- one-time setup ----------------
            id_bf = const_pool.tile([128, 128], BF16)
            masks.make_identity(nc, id_bf[:, :])

            # Natural-layout weights, cast to bf16 during DMA.
            Wq_sb = const_pool.tile([128, 2, D], BF16)  # [a_lo, ac, qd]
            nc.gpsimd.dma_start(
                Wq_sb[:, :, :], Wq.rearrange("(ac p) q -> p ac q", p=128)
            )
            Wv_sb = const_pool.tile([128, 2, D], BF16)  # [a_lo, vc, v]
            nc.gpsimd.dma_start(
                Wv_sb[:, :, :], Wv.rearrange("(ac p) q -> p ac q", p=128)
            )
            # query^T gather: [qd_lo, qc, b]
            qT_sb = const_pool.tile([128, 2, B_LOC], BF16)
            for qc in range(2):
                nc.gpsimd.dma_start(
                    qT_sb[:, qc, :],
                    query[:, 128 * qc : 128 * (qc + 1)].rearrange("b p -> p b"),
                )
            # bv, bq per-partition: [a_lo, ac]
            bv_sb = const_pool.tile([128, 2], F32)
            nc.sync.dma_start(bv_sb[:, :], bv.rearrange("(ac p) -> p ac", p=128))
            bq_sb = const_pool.tile([128, 2], F32)
            nc.sync.dma_start(bq_sb[:, :], bq.rearrange("(ac p) -> p ac", p=128))
            # wv_score per-partition bf16: [a_lo, ac]
            wv_sb = const_pool.tile([128, 2], BF16)
            nc.gpsimd.dma_start(
                wv_sb[:, :], wv_score.rearrange("(ac p) -> p ac", p=128)
            )

            # Transposed weights via PE: WqT[qd, a], WvT[v, a]
            WqT_sb = const_pool.tile([128, 2, D], BF16)  # [qd_lo, qc, a]
            WvT_sb = const_pool.tile([128, 2, D], BF16)  # [v_lo, vc, a]
            for (w_sb, wT_sb) in ((Wq_sb, WqT_sb), (Wv_sb, WvT_sb)):
                for qc in range(2):
                    wt_ps = ps_vt.tile([128, 256], BF16, tag="vt")
                    for ac in range(2):
                        nc.tensor.transpose(
                            wt_ps[:, 128 * ac : 128 * (ac + 1)],
                            w_sb[:, ac, 128 * qc : 128 * (qc + 1)],
                            id_bf[:, :],
                        )
                    nc.vector.tensor_copy(wT_sb[:, qc, :], wt_ps[:, :])

            # Band of wv_score for the one-hot e-matmul:
            # W_band[p, ac, col] = wv_score[128*ac + p] iff col == 128.
            # lhsT slice [128, 128] starting at col 128-j selects output row j.
            W_band = const_pool.tile([128, 2, 2 * 128], BF16)
            nc.vector.memset(W_band[:, :, :], 0.0)
            for ac in range(2):
                nc.vector.tensor_copy(W_band[:, ac, 128:129], wv_sb[:, ac : ac + 1])

            # ones for the per-batch sum-of-8-rows replication matmul
            ones8 = const_pool.tile([128, 8], F32)
            nc.vector.memset(ones8[:, :], 1.0)

            # persistent small tiles
            a_unnorm = const_pool.tile([128, 512], BF16)
            nc.vector.memset(a_unnorm[:, :], 0.0)
            a_norm = const_pool.tile([128, 512], BF16)
            nc.vector.memset(a_norm[:, :], 0.0)
            accum_sb = const_pool.tile([128, 1], F32)
            nc.vector.memset(accum_sb[:, :], 0.0)
            recip_sb = const_pool.tile([128, 1], F32)
            c_sb = const_pool.tile([128, D], F32)

            # q projection: q_ps[a_lo, ac*4 + b] = (Wq @ query_b)[a]
            q_ps = ps_c.tile([128, 2 * B_LOC], F32, tag="csums")
            for ac in range(2):
                for qc in range(2):
                    nc.tensor.matmul(
                        q_ps[:, B_LOC * ac : B_LOC * (ac + 1)],
                        WqT_sb[:, qc, 128 * ac : 128 * (ac + 1)],
                        qT_sb[:, qc, :],
                        start=(qc == 0),
                        stop=(qc == 1),
                    )
            # qbias = q + bq + bv  (tanh bias, per-partition)
            qbias = const_pool.tile([128, 2 * B_LOC], F32)
            for ac in range(2):
                nc.vector.tensor_scalar(
                    qbias[:, B_LOC * ac : B_LOC * (ac + 1)],
                    q_ps[:, B_LOC * ac : B_LOC * (ac + 1)],
                    bq_sb[:, ac : ac + 1],
                    bv_sb[:, ac : ac + 1],
                    op0=mybir.AluOpType.add,
                    op1=mybir.AluOpType.add,
                )

            # e accumulator: row 32*b + s holds e[b, 512*s : 512*(s+1)]
            e_ps = ps_e.tile([128, 512], F32, tag="e")
            # c accumulator + per-batch sums: cols 0..255 = c, col 256 = sum
            cs_ps = ps_c.tile([128, D + 8], F32, tag="csums")

            n_emm = 0

            # ---------------- main per-batch pipeline ----------------
            for b in range(B_LOC):
                v_nat = vnat_pool.tile([128, T // 128, D], BF16, tag="vnat")
                vb = values[b, :, :]  # [T, D] dram AP
                for g in range(4):  # 1 MiB (f32) per DMA
                    nc.gpsimd.dma_start(
                        v_nat[:, 8 * g : 8 * (g + 1), :],
                        vb[1024 * g : 1024 * (g + 1), :].rearrange(
                            "(c p) v -> p c v", p=128
                        ),
                    )

                for g in range(4):  # t groups of 1024
                    vT_sbs = []
                    for vc in range(2):
                        vT_ps = ps_vt.tile([128, 1024], BF16, tag="vt")
                        for cc in range(8):
                            nc.tensor.transpose(
                                vT_ps[:, 128 * cc : 128 * (cc + 1)],
                                v_nat[:, 8 * g + cc, 128 * vc : 128 * (vc + 1)],
                                id_bf[:, :],
                            )
                        vT_sb = vtsb_pool.tile([128, 1024], BF16, tag="vtsb")
                        nc.vector.tensor_copy(vT_sb[:, :], vT_ps[:, :])
                        vT_sbs.append(vT_sb)

                    for ac in range(2):
                        k_ps = ps_keys.tile([128, 1024], F32, tag="keys")
                        for vc in range(2):
                            for h in range(2):
                                nc.tensor.matmul(
                                    k_ps[:, 512 * h : 512 * (h + 1)],
                                    WvT_sb[:, vc, 128 * ac : 128 * (ac + 1)],
                                    vT_sbs[vc][:, 512 * h : 512 * (h + 1)],
                                    start=(vc == 0),
                                    stop=(vc == 1),
                                )
                        tanhT = tanh_pool.tile([128, 1024], BF16, tag="tanh")
                        nc.scalar.activation(
                            tanhT[:, :],
                            k_ps[:, :],
                            AF.Tanh,
                            bias=qbias[:, B_LOC * ac + b : B_LOC * ac + b + 1],
                        )
                        for h in range(2):
                            s = 2 * g + h
                            j = 32 * b + s
                            nc.tensor.matmul(
                                e_ps[:, :],
                                W_band[:, ac, 128 - j : 256 - j],
                                tanhT[:, 512 * h : 512 * (h + 1)],
                                start=(n_emm == 0),
                                stop=(n_emm == 4 * B_LOC * 4 - 1),
                                skip_group_check=True,
                            )
                            n_emm += 1

                # softmax (no max subtraction; |e| is small) + denominator
                r0 = 32 * b
                nc.scalar.activation(
                    a_unnorm[r0 : r0 + 8, :],
                    e_ps[r0 : r0 + 8, :],
                    AF.Exp,
                    accum_out=accum_sb[r0 : r0 + 8, :],
                )
                # replicate sum of the 8 slice-rows to all 8 rows: ones8.T @ accum
                nc.tensor.matmul(
                    cs_ps[r0 : r0 + 8, D : D + 1],
                    ones8[r0 : r0 + 8, :],
                    accum_sb[r0 : r0 + 8, :],
                    start=True,
                    stop=True,
                    skip_group_check=True,
                    tile_position=(r0, r0),
                )
                nc.vector.reciprocal(
                    recip_sb[r0 : r0 + 8, :], cs_ps[r0 : r0 + 8, D : D + 1]
                )
                nc.vector.tensor_scalar_mul(
                    a_norm[r0 : r0 + 8, :],
                    a_unnorm[r0 : r0 + 8, :],
                    recip_sb[r0 : r0 + 8, 0:1],
                )
                # stream normalized attention out (bf16 -> f32 cast DMA)
                nc.gpsimd.dma_start(
                    a_out[b].rearrange("(s u) -> s u", s=8), a_norm[r0 : r0 + 8, :]
                )

                # transpose a_norm into [t_lo, 32*cb + s] columns for the c matmul
                aT_ps = ps_vt.tile([128, 1024], BF16, tag="vt")
                for cb in range(4):
                    nc.tensor.transpose(
                        aT_ps[:, 32 * cb : 32 * (cb + 1)],
                        a_norm[r0 : r0 + 32, 128 * cb : 128 * (cb + 1)],
                        id_bf[r0 : r0 + 32, r0 : r0 + 32],
                        tile_position=(r0, 0),
                    )
                aT_sb = atsb_pool.tile([128, 128], BF16, tag="atsb")
                nc.vector.tensor_copy(aT_sb[:, :], aT_ps[:, 0:128])

                # c[b] = a_norm[b] @ values[b] accumulated over 32 t-chunks
                for s in range(8):
                    for cb in range(4):
                        chunk = 4 * s + cb
                        nc.tensor.matmul(
                            cs_ps[r0 : r0 + 1, 0:D],
                            aT_sb[:, 32 * cb + s : 32 * cb + s + 1],
                            v_nat[:, chunk, :],
                            start=(chunk == 0),
                            stop=(chunk == 31),
                            skip_group_check=True,
                            tile_position=(0, r0),
                        )
                nc.vector.tensor_copy(c_sb[r0 : r0 + 1, :], cs_ps[r0 : r0 + 1, 0:D])

            nc.sync.dma_start(c_out[:, :], c_sb[0:128:32, :])

    nc.compile()
    _NC_CACHE = nc
    return nc


def make_in_maps(query, values, Wq, bq, Wv, bv, wv_score, **_ignored):
    def f32(x):
        return np.ascontiguousarray(np.asarray(x, dtype=np.float32))

    maps = []
    for i in range(N_CORES):
        sl = slice(B_LOC * i, B_LOC * (i + 1))
        maps.append(
            {
                "values": f32(values[sl]),
                "query": f32(query[sl]),
                "Wq": f32(Wq),
                "bq": f32(bq),
                "Wv": f32(Wv),
                "bv": f32(bv),
                "wv_score": f32(wv_score),
            }
        )
    return maps


def kernel(**inputs):
    nc = build_nc()
    in_maps = make_in_maps(**inputs)
    res = run_bass_kernel_spmd(nc, in_maps, core_ids=list(range(N_CORES)))
    c = np.concatenate([res.results[i]["c_out"] for i in range(N_CORES)], axis=0)
    a = np.concatenate([res.results[i]["a_out"] for i in range(N_CORES)], axis=0)
    return (c, a)


# revision 15
# speedup vs baseline: 1.0156x; 1.0156x over previous
"""Bahdanau attention mechanism on 8 Trainium2 NeuronCores.

Problem (full shapes): B=32, T=4096, QD=VD=AD=256
    keys = values @ Wv.T + bv            [B, T, AD]
    q    = query @ Wq.T + bq             [B, AD]
    e    = tanh(q[:,None,:] + keys) @ wv_score + bv_score   [B, T]
    a    = softmax(e, axis=-1)           [B, T]
    c    = einsum('bt,btv->bv', a, values)

Sharding: data-parallel over batch. Each of the 8 cores gets 4 batches;
all weights replicated. No collectives.

Per-core dataflow (all matmul inputs bf16, fp32 accumulation):
  - values are DMA'd HBM->SBUF with an f32->bf16 cast (SWDGE).
  - PE transposes 128x128 blocks of values to get v^T for the keys matmul
    (bf16 transposes write bf16 PSUM, evacuated by DVE at 2x rate).
  - keys^T[a, t] accumulates in PSUM; ACT applies tanh with the per-partition
    bias q[a]+bv[a] and writes bf16 tanh^T tiles.
  - e is reduced over a with a matmul against a one-hot-shifted "band" of
    wv_score so that each 512-wide t-slice of e lands in its own PSUM row
    (rows 32b+s, all contiguous) - avoids M=1 output-base restrictions.
  - exp (no max subtraction: |e| <= ~4 so exp is safe in fp32) with
    accum_out produces the softmax denominator for free.
  - a is normalized on DVE, streamed out with a bf16->f32 cast DMA, and
    transposed on PE ([32,128] blocks) to feed the c matmul.
  - c[b] = a_norm @ values accumulates in PSUM row 32b and is copied out.

bv_score is mathematically irrelevant (softmax shift invariance) and the
outputs (c, a) do not depend on it, so the kernel ignores it.
"""

import numpy as np

import concourse.bacc as bacc
import concourse.mybir as mybir
import concourse.tile as tile
from concourse import masks
from concourse.bass_utils import run_bass_kernel_spmd

N_CORES = 8
B = 32
B_LOC = B // N_CORES  # 4
T = 4096
D = 256  # QD == VD == AD

F32 = mybir.dt.float32
BF16 = mybir.dt.bfloat16
AF = mybir.ActivationFunctionType

_NC_CACHE = None


def build_nc(
    vnat_bufs=2,
    vtsb_bufs=4,
    tanh_bufs=3,
    vt_ps_bufs=2,
    keys_ps_bufs=2,
    skip_c=False,
    skip_e=False,
    skip_keys=False,
    cache=True,
):
    """Build (and cache) the per-core Bass program."""
    global _NC_CACHE
    if cache and _NC_CACHE is not None:
        return _NC_CACHE

    nc = bacc.Bacc(
        "TRN2", target_bir_lowering=False, debug=False, num_devices=N_CORES
    )

    values = nc.dram_tensor("values", [B_LOC, T, D], F32, kind="ExternalInput").ap()
    query = nc.dram_tensor("query", [B_LOC, D], F32, kind="ExternalInput").ap()
    Wq = nc.dram_tensor("Wq", [D, D], F32, kind="ExternalInput").ap()
    bq = nc.dram_tensor("bq", [D], F32, kind="ExternalInput").ap()
    Wv = nc.dram_tensor("Wv", [D, D], F32, kind="ExternalInput").ap()
    bv = nc.dram_tensor("bv", [D], F32, kind="ExternalInput").ap()
    wv_score = nc.dram_tensor("wv_score", [D], F32, kind="ExternalInput").ap()
    c_out = nc.dram_tensor("c_out", [B_LOC, D], F32, kind="ExternalOutput").ap()
    a_out = nc.dram_tensor("a_out", [B_LOC, T], F32, kind="ExternalOutput").ap()

    with tile.TileContext(nc) as tc:
        with (
            tc.tile_pool(name="const", bufs=1) as const_pool,
            tc.tile_pool(name="vnat", bufs=vnat_bufs) as vnat_pool,
            tc.tile_pool(name="vtsb", bufs=vtsb_bufs) as vtsb_pool,
            tc.tile_pool(name="tanh", bufs=tanh_bufs) as tanh_pool,
            tc.tile_pool(name="atsb", bufs=2) as atsb_pool,
            tc.tile_pool(name="ps_vt", bufs=vt_ps_bufs, space="PSUM") as ps_vt,
            tc.tile_pool(name="ps_keys", bufs=keys_ps_bufs, space="PSUM") as ps_keys,
            tc.tile_pool(name="ps_e", bufs=1, space="PSUM") as ps_e,  # 1 bank
            tc.tile_pool(name="ps_c", bufs=1, space="PSUM") as ps_c,  # 1 bank
        ):
            # ---------------- one-time setup ----------------
            id_bf = const_pool.tile([128, 128], BF16)
            masks.make_identity(nc, id_bf[:, :])
            id_f32 = const_pool.tile([128, 128], F32)
            masks.make_identity(nc, id_f32[:, :])

            # Natural-layout weights, cast to bf16 during DMA.
            Wq_sb = const_pool.tile([128, 2, D], BF16)  # [a_lo, ac, qd]
            nc.gpsimd.dma_start(
                Wq_sb[:, :, :], Wq.rearrange("(ac p) q -> p ac q", p=128)
            )
            Wv_sb = const_pool.tile([128, 2, D], BF16)  # [a_lo, vc, v]
            nc.gpsimd.dma_start(
                Wv_sb[:, :, :], Wv.rearrange("(ac p) q -> p ac q", p=128)
            )
            # query^T gather: [qd_lo, qc, b]
            qT_sb = const_pool.tile([128, 2, B_LOC], BF16)
            for qc in range(2):
                nc.gpsimd.dma_start(
                    qT_sb[:, qc, :],
                    query[:, 128 * qc : 128 * (qc + 1)].rearrange("b p -> p b"),
                )
            # bv, bq per-partition: [a_lo, ac]
            bv_sb = const_pool.tile([128, 2], F32)
            nc.sync.dma_start(bv_sb[:, :], bv.rearrange("(ac p) -> p ac", p=128))
            bq_sb = const_pool.tile([128, 2], F32)
            nc.sync.dma_start(bq_sb[:, :], bq.rearrange("(ac p) -> p ac", p=128))
            # wv_score per-partition bf16: [a_lo, ac]
            wv_sb = const_pool.tile([128, 2], BF16)
            nc.gpsimd.dma_start(
                wv_sb[:, :], wv_score.rearrange("(ac p) -> p ac", p=128)
            )

            # Transposed weights via PE: WqT[qd, a], WvT[v, a]
            WqT_sb = const_pool.tile([128, 2, D], BF16)  # [qd_lo, qc, a]
            WvT_sb = const_pool.tile([128, 2, D], BF16)  # [v_lo, vc, a]
            for (w_sb, wT_sb) in ((Wq_sb, WqT_sb), (Wv_sb, WvT_sb)):
                for qc in range(2):
                    wt_ps = ps_vt.tile([128, 256], BF16, tag="vt")
                    for ac in range(2):
                        nc.tensor.transpose(
                            wt_ps[:, 128 * ac : 128 * (ac + 1)],
                            w_sb[:, ac, 128 * qc : 128 * (qc + 1)],
                            id_bf[:, :],
                        )
                    nc.vector.tensor_copy(wT_sb[:, qc, :], wt_ps[:, :])

            # Band of wv_score for the one-hot e-matmul:
            # W_band[p, ac, col] = wv_score[128*ac + p] iff col == 128.
            # lhsT slice [128, 128] starting at col 128-j selects output row j.
            W_band = const_pool.tile([128, 2, 2 * 128], BF16)
            nc.vector.memset(W_band[:, :, :], 0.0)
            for ac in range(2):
                nc.vector.tensor_copy(W_band[:, ac, 128:129], wv_sb[:, ac : ac + 1])

            # ones for the per-batch sum-of-8-rows replication matmul
            ones8 = const_pool.tile([128, 8], F32)
            nc.vector.memset(ones8[:, :], 1.0)

            # persistent small tiles
            a_unnorm = const_pool.tile([128, 512], BF16)
            nc.vector.memset(a_unnorm[:, :], 0.0)
            a_norm = const_pool.tile([128, 512], BF16)
            nc.vector.memset(a_norm[:, :], 0.0)
            accum_sb = const_pool.tile([128, 1], F32)
            nc.vector.memset(accum_sb[:, :], 0.0)
            recip_sb = const_pool.tile([128, 1], F32)
            nc.vector.memset(recip_sb[:, :], 0.0)
            c_sb = const_pool.tile([128, B_LOC * D], F32)
            nc.vector.memset(c_sb[:, :], 0.0)
            cT_sb = const_pool.tile([128, B_LOC, 2], F32)

            # q projection: q_ps[a_lo, ac*4 + b] = (Wq @ query_b)[a]
            q_ps = ps_c.tile([128, 2 * B_LOC], F32, tag="csums")
            for ac in range(2):
                for qc in range(2):
                    nc.tensor.matmul(
                        q_ps[:, B_LOC * ac : B_LOC * (ac + 1)],
                        WqT_sb[:, qc, 128 * ac : 128 * (ac + 1)],
                        qT_sb[:, qc, :],
                        start=(qc == 0),
                        stop=(qc == 1),
                    )
            # qbias = q + bq + bv  (tanh bias, per-partition)
            qbias = const_pool.tile([128, 2 * B_LOC], F32)
            for ac in range(2):
                nc.vector.tensor_scalar(
                    qbias[:, B_LOC * ac : B_LOC * (ac + 1)],
                    q_ps[:, B_LOC * ac : B_LOC * (ac + 1)],
                    bq_sb[:, ac : ac + 1],
                    bv_sb[:, ac : ac + 1],
                    op0=mybir.AluOpType.add,
                    op1=mybir.AluOpType.add,
                )

            # e accumulator: row 32*b + s holds e[b, 512*s : 512*(s+1)]
            e_ps = ps_e.tile([128, 512], F32, tag="e")
            # c accumulator + per-batch sums: cols 0..255 = c, col 256 = sum
            cs_ps = ps_c.tile([128, D + 8], F32, tag="csums")

            n_emm = 0

            # ---------------- main per-batch pipeline ----------------
            for b in range(B_LOC):
                v_nat = vnat_pool.tile([128, T // 128, D], BF16, tag="vnat")
                vb = values[b, :, :]  # [T, D] dram AP
                for g in range(4):  # 1 MiB (f32) per DMA
                    nc.gpsimd.dma_start(
                        v_nat[:, 8 * g : 8 * (g + 1), :],
                        vb[1024 * g : 1024 * (g + 1), :].rearrange(
                            "(c p) v -> p c v", p=128
                        ),
                    )

                for g in range(4):  # t groups of 1024
                    vT_sbs = []
                    for vc in range(2):
                        vT_ps = ps_vt.tile([128, 1024], BF16, tag="vt")
                        for cc in range(8):
                            nc.tensor.transpose(
                                vT_ps[:, 128 * cc : 128 * (cc + 1)],
                                v_nat[:, 8 * g + cc, 128 * vc : 128 * (vc + 1)],
                                id_bf[:, :],
                            )
                        vT_sb = vtsb_pool.tile([128, 1024], BF16, tag="vtsb")
                        nc.vector.tensor_copy(vT_sb[:, :], vT_ps[:, :])
                        vT_sbs.append(vT_sb)

                    for ac in range(2):
                        if skip_keys:
                            break
                        k_ps = ps_keys.tile([128, 1024], F32, tag="keys")
                        for vc in range(2):
                            for h in range(2):
                                nc.tensor.matmul(
                                    k_ps[:, 512 * h : 512 * (h + 1)],
                                    WvT_sb[:, vc, 128 * ac : 128 * (ac + 1)],
                                    vT_sbs[vc][:, 512 * h : 512 * (h + 1)],
                                    start=(vc == 0),
                                    stop=(vc == 1),
                                )
                        tanhT = tanh_pool.tile([128, 1024], BF16, tag="tanh")
                        nc.scalar.activation(
                            tanhT[:, :],
                            k_ps[:, :],
                            AF.Tanh,
                            bias=qbias[:, B_LOC * ac + b : B_LOC * ac + b + 1],
                        )
                        for h in range(2):
                            if skip_e:
                                break
                            s = 2 * g + h
                            j = 32 * b + s
                            nc.tensor.matmul(
                                e_ps[:, :],
                                W_band[:, ac, 128 - j : 256 - j],
                                tanhT[:, 512 * h : 512 * (h + 1)],
                                start=(n_emm == 0),
                                stop=(n_emm == 4 * B_LOC * 4 - 1),
                                skip_group_check=True,
                            )
                            n_emm += 1

                # softmax (no max subtraction; |e| is small) + denominator
                r0 = 32 * b
                if skip_keys or skip_e:
                    continue
                nc.scalar.activation(
                    a_unnorm[r0 : r0 + 8, :],
                    e_ps[r0 : r0 + 8, :],
                    AF.Exp,
                    accum_out=accum_sb[r0 : r0 + 8, :],
                )
                # replicate sum of the 8 slice-rows to all 8 rows: ones8.T @ accum
                nc.tensor.matmul(
                    cs_ps[r0 : r0 + 8, D : D + 1],
                    ones8[r0 : r0 + 8, :],
                    accum_sb[r0 : r0 + 8, :],
                    start=True,
                    stop=True,
                    skip_group_check=True,
                    tile_position=(r0, r0),
                )
                nc.vector.reciprocal(
                    recip_sb[r0 : r0 + 8, :], cs_ps[r0 : r0 + 8, D : D + 1]
                )
                nc.vector.tensor_scalar_mul(
                    a_norm[r0 : r0 + 8, :],
                    a_unnorm[r0 : r0 + 8, :],
                    recip_sb[r0 : r0 + 8, 0:1],
                )
                # stream normalized attention out (bf16 -> f32 cast DMA)
                nc.gpsimd.dma_start(
                    a_out[b].rearrange("(s u) -> s u", s=8), a_norm[r0 : r0 + 8, :]
                )

                # transpose a_norm into [t_lo, 32*cb + s] columns for the c matmul
                if skip_c:
                    continue
                aT_ps = ps_vt.tile([128, 1024], BF16, tag="vt")
                for cb in range(4):
                    nc.tensor.transpose(
                        aT_ps[:, 32 * cb : 32 * (cb + 1)],
                        a_norm[r0 : r0 + 32, 128 * cb : 128 * (cb + 1)],
                        id_bf[r0 : r0 + 32, r0 : r0 + 32],
                        tile_position=(r0, 0),
                    )
                aT_sb = atsb_pool.tile([128, 128], BF16, tag="atsb")
                nc.vector.tensor_copy(aT_sb[:, :], aT_ps[:, 0:128])

                # c[b] = a_norm[b] @ values[b]: 4 concurrent 32-col PE tiles,
                # col-group j handles s in {2j, 2j+1}; partial lands in PSUM
                # row 32j.  Partials are combined by accumulate-DMAs into the
                # zero-initialized c_out DRAM buffer.
                # round-robin across the 4 col-tiles so they run concurrently
                for m in range(8):
                    for j in range(4):
                        s = 2 * j + m // 4
                        cb = m % 4
                        chunk = 4 * s + cb
                        nc.tensor.matmul(
                            cs_ps[32 * j : 32 * j + 1, 0:D],
                            aT_sb[:, 32 * cb + s : 32 * cb + s + 1],
                            v_nat[:, chunk, :],
                            start=(m == 0),
                            stop=(m == 7),
                            skip_group_check=True,
                            tile_position=(0, 32 * j),
                        )
                for j in range(4):
                    nc.vector.tensor_copy(
                        c_sb[32 * j : 32 * j + 1, D * b : D * (b + 1)],
                        cs_ps[32 * j : 32 * j + 1, 0:D],
                    )
                # combine the 4 partial rows: PE-transpose the [128, 256]
                # block (partials are rows {0,32,64,96}) and reduce the
                # strided columns on DVE -> c^T[v] columns in cT_sb.
                cT_ps = ps_vt.tile([128, 256], F32, tag="vt")
                for vc in range(2):
                    nc.tensor.transpose(
                        cT_ps[:, 128 * vc : 128 * (vc + 1)],
                        c_sb[:, D * b + 128 * vc : D * b + 128 * (vc + 1)],
                        id_f32[:, :],
                    )
                for vc in range(2):
                    nc.vector.reduce_sum(
                        cT_sb[:, b, vc : vc + 1],
                        cT_ps[:, 128 * vc : 128 * vc + 97 : 32],
                        axis=mybir.AxisListType.X,
                    )

            nc.sync.dma_start(
                c_out.rearrange("b (vc p) -> p b vc", p=128), cT_sb[:, :, :]
            )

    nc.compile()
    if cache:
        _NC_CACHE = nc
    return nc


def make_in_maps(query, values, Wq, bq, Wv, bv, wv_score, **_ignored):
    def f32(x):
        return np.ascontiguousarray(np.asarray(x, dtype=np.float32))

    maps = []
    for i in range(N_CORES):
        sl = slice(B_LOC * i, B_LOC * (i + 1))
        maps.append(
            {
                "values": f32(values[sl]),
                "query": f32(query[sl]),
                "Wq": f32(Wq),
                "bq": f32(bq),
                "Wv": f32(Wv),
                "bv": f32(bv),
                "wv_score": f32(wv_score),
            }
        )
    return maps


def kernel(**inputs):
    nc = build_nc()
    in_maps = make_in_maps(**inputs)
    res = run_bass_kernel_spmd(nc, in_maps, core_ids=list(range(N_CORES)))
    c = np.concatenate([res.results[i]["c_out"] for i in range(N_CORES)], axis=0)
    a = np.concatenate([res.results[i]["a_out"] for i in range(N_CORES)], axis=0)
    return (c, a)


# revision 26
# speedup vs baseline: 45.6113x; 44.9117x over previous
"""Bahdanau attention mechanism on 8 Trainium2 NeuronCores.

Problem (full shapes): B=32, T=4096, QD=VD=AD=256
    keys = values @ Wv.T + bv            [B, T, AD]
    q    = query @ Wq.T + bq             [B, AD]
    e    = tanh(q[:,None,:] + keys) @ wv_score + bv_score   [B, T]
    a    = softmax(e, axis=-1)           [B, T]
    c    = einsum('bt,btv->bv', a, values)

Sharding: data-parallel over batch. Each of the 8 cores gets 4 batches;
all weights replicated. No collectives.

Per-core dataflow (all matmul inputs bf16, fp32 accumulation):
  - values are DMA'd HBM->SBUF with an f32->bf16 cast (SWDGE).
  - PE transposes 128x128 blocks of values to get v^T for the keys matmul
    (bf16 transposes write bf16 PSUM, evacuated by DVE at 2x rate).
  - keys^T[a, t] accumulates in PSUM; ACT applies tanh with the per-partition
    bias q[a]+bv[a] and writes bf16 tanh^T tiles.
  - e is reduced over a with a matmul against a one-hot-shifted "band" of
    wv_score so that each 512-wide t-slice of e lands in its own PSUM row
    (rows 32b+s, all contiguous) - avoids M=1 output-base restrictions.
  - exp (no max subtraction: |e| <= ~4 so exp is safe in fp32) with
    accum_out produces the softmax denominator for free.
  - a is normalized on DVE, streamed out with a bf16->f32 cast DMA, and
    transposed on PE ([32,128] blocks) to feed the c matmul.
  - c[b] = a_norm @ values accumulates in PSUM row 32b and is copied out.

bv_score is mathematically irrelevant (softmax shift invariance) and the
outputs (c, a) do not depend on it, so the kernel ignores it.
"""

import numpy as np

import concourse.bacc as bacc
import concourse.mybir as mybir
import concourse.tile as tile
from concourse import masks
from concourse.bass_utils import run_bass_kernel_spmd

N_CORES = 8
B = 32
B_LOC = B // N_CORES  # 4
T = 4096
D = 256  # QD == VD == AD

F32 = mybir.dt.float32
BF16 = mybir.dt.bfloat16
AF = mybir.ActivationFunctionType

_NC_CACHE = None


def build_nc(
    vnat_bufs=3,
    vtsb_bufs=4,
    tanh_bufs=3,
    vt_ps_bufs=2,
    keys_ps_bufs=2,
    tail_lag=1,
    reps=1,
    ct_c=True,
    skip_c=False,
    skip_e=False,
    skip_keys=False,
    cache=True,
):
    """Build (and cache) the per-core Bass program."""
    global _NC_CACHE
    if cache and _NC_CACHE is not None:
        return _NC_CACHE

    nc = bacc.Bacc(
        "TRN2", target_bir_lowering=False, debug=False, num_devices=N_CORES
    )

    values = nc.dram_tensor("values", [B_LOC, T, D], F32, kind="ExternalInput").ap()
    query = nc.dram_tensor("query", [B_LOC, D], F32, kind="ExternalInput").ap()
    Wq = nc.dram_tensor("Wq", [D, D], F32, kind="ExternalInput").ap()
    bq = nc.dram_tensor("bq", [D], F32, kind="ExternalInput").ap()
    Wv = nc.dram_tensor("Wv", [D, D], F32, kind="ExternalInput").ap()
    bv = nc.dram_tensor("bv", [D], F32, kind="ExternalInput").ap()
    wv_score = nc.dram_tensor("wv_score", [D], F32, kind="ExternalInput").ap()
    c_out = nc.dram_tensor("c_out", [B_LOC, D], F32, kind="ExternalOutput").ap()
    a_out = nc.dram_tensor("a_out", [B_LOC, T], F32, kind="ExternalOutput").ap()

    with tile.TileContext(nc) as tc:
        with (
            tc.tile_pool(name="const", bufs=1) as const_pool,
            tc.tile_pool(name="vnat", bufs=vnat_bufs) as vnat_pool,
            tc.tile_pool(name="vtsb", bufs=vtsb_bufs) as vtsb_pool,
            tc.tile_pool(name="tanh", bufs=tanh_bufs) as tanh_pool,
            tc.tile_pool(name="atsb", bufs=2) as atsb_pool,
            tc.tile_pool(name="ps_vt", bufs=vt_ps_bufs, space="PSUM") as ps_vt,
            tc.tile_pool(name="ps_keys", bufs=keys_ps_bufs, space="PSUM") as ps_keys,
            tc.tile_pool(name="ps_e", bufs=1, space="PSUM") as ps_e,  # 1 bank
            tc.tile_pool(name="ps_c", bufs=1, space="PSUM") as ps_c,  # 1 bank
        ):
            # ---------------- one-time setup ----------------
            # Identity first (GPSIMD), then immediately queue batch 0's big
            # values DMAs so the SWDGE engine and DMA queues start streaming;
            # everything else loads via HWDGE (no GPSIMD descriptor time).
            id_bf = const_pool.tile([128, 128], BF16)
            masks.make_identity(nc, id_bf[:, :])

            v_nats = {}

            def prefetch(r, b, split_first=False):
                v_nat = vnat_pool.tile([128, T // 128, D], BF16, tag="vnat")
                v_nats[(r, b)] = v_nat
                vb = values[b, :, :]  # [T, D] dram AP
                vb3 = vb.rearrange("(c p) v -> p c v", p=128)
                if split_first:
                    # smaller leading transfers so the first transposes can
                    # start as early as possible
                    for cc in range(4):
                        nc.gpsimd.dma_start(
                            v_nat[:, 2 * cc : 2 * (cc + 1), :],
                            vb3[:, 2 * cc : 2 * (cc + 1), :],
                        )
                    gs = range(1, 4)
                else:
                    gs = range(4)
                for g in gs:  # 1 MiB (f32) per DMA
                    nc.gpsimd.dma_start(
                        v_nat[:, 8 * g : 8 * (g + 1), :],
                        vb3[:, 8 * g : 8 * (g + 1), :],
                    )

            prefetch(0, 0, split_first=True)

            # Natural-layout weights: HWDGE f32 load + DVE cast to bf16.
            w_stage = const_pool.tile([128, 2, D], F32)
            Wq_sb = const_pool.tile([128, 2, D], BF16)  # [a_lo, ac, qd]
            nc.sync.dma_start(
                w_stage[:, :, :], Wq.rearrange("(ac p) q -> p ac q", p=128)
            )
            nc.vector.tensor_copy(Wq_sb[:, :, :], w_stage[:, :, :])
            w2_stage = const_pool.tile([128, 2, D], F32)
            Wv_sb = const_pool.tile([128, 2, D], BF16)  # [a_lo, vc, v]
            nc.sync.dma_start(
                w2_stage[:, :, :], Wv.rearrange("(ac p) q -> p ac q", p=128)
            )
            nc.vector.tensor_copy(Wv_sb[:, :, :], w2_stage[:, :, :])
            # query^T gather: [qd_lo, qc, b]
            qstage = const_pool.tile([128, 2, B_LOC], F32)
            qT_sb = const_pool.tile([128, 2, B_LOC], BF16)
            for qc in range(2):
                nc.sync.dma_start(
                    qstage[:, qc, :],
                    query[:, 128 * qc : 128 * (qc + 1)].rearrange("b p -> p b"),
                )
            nc.vector.tensor_copy(qT_sb[:, :, :], qstage[:, :, :])
            # bv, bq per-partition: [a_lo, ac]
            bv_sb = const_pool.tile([128, 2], F32)
            nc.sync.dma_start(bv_sb[:, :], bv.rearrange("(ac p) -> p ac", p=128))
            bq_sb = const_pool.tile([128, 2], F32)
            nc.sync.dma_start(bq_sb[:, :], bq.rearrange("(ac p) -> p ac", p=128))
            # wv_score per-partition bf16: [a_lo, ac]
            wstage = const_pool.tile([128, 2], F32)
            wv_sb = const_pool.tile([128, 2], BF16)
            nc.sync.dma_start(
                wstage[:, :], wv_score.rearrange("(ac p) -> p ac", p=128)
            )
            nc.vector.tensor_copy(wv_sb[:, :], wstage[:, :])
            id_f32 = const_pool.tile([128, 128], F32)
            masks.make_identity(nc, id_f32[:, :])

            # Transposed weights via PE: WqT[qd, a], WvT[v, a]
            WqT_sb = const_pool.tile([128, 2, D], BF16)  # [qd_lo, qc, a]
            WvT_sb = const_pool.tile([128, 2, D], BF16)  # [v_lo, vc, a]
            for (w_sb, wT_sb) in ((Wq_sb, WqT_sb), (Wv_sb, WvT_sb)):
                for qc in range(2):
                    wt_ps = ps_vt.tile([128, 256], BF16, tag="vt")
                    for ac in range(2):
                        nc.tensor.transpose(
                            wt_ps[:, 128 * ac : 128 * (ac + 1)],
                            w_sb[:, ac, 128 * qc : 128 * (qc + 1)],
                            id_bf[:, :],
                        )
                    nc.vector.tensor_copy(wT_sb[:, qc, :], wt_ps[:, :])

            # Band of wv_score for the one-hot e-matmul:
            # W_band[p, ac, col] = wv_score[128*ac + p] iff col == 128.
            # lhsT slice [128, 128] starting at col 128-j selects output row j.
            W_band = const_pool.tile([128, 2, 2 * 128], BF16)
            nc.vector.memset(W_band[:, :, :], 0.0)
            for ac in range(2):
                nc.vector.tensor_copy(W_band[:, ac, 128:129], wv_sb[:, ac : ac + 1])

            # ones for the per-batch sum-of-8-rows replication matmul
            ones8 = const_pool.tile([128, 8], F32)
            nc.vector.memset(ones8[:, :], 1.0)

            # persistent small tiles
            a_unnorm = const_pool.tile([128, 512], BF16)
            nc.vector.memset(a_unnorm[:, :], 0.0)
            a_norm = const_pool.tile([128, 512], BF16)
            nc.vector.memset(a_norm[:, :], 0.0)
            accum_sb = const_pool.tile([128, 1], F32)
            nc.vector.memset(accum_sb[:, :], 0.0)
            recip_sb = const_pool.tile([128, 1], F32)
            nc.vector.memset(recip_sb[:, :], 0.0)
            c_sb = const_pool.tile([128, B_LOC * D], F32)
            nc.vector.memset(c_sb[:, :], 0.0)
            cT_sb = const_pool.tile([128, B_LOC, 2], F32)

            # q projection: q_ps[a_lo, ac*4 + b] = (Wq @ query_b)[a]
            q_ps = ps_c.tile([128, 2 * B_LOC], F32, tag="csums")
            for ac in range(2):
                for qc in range(2):
                    nc.tensor.matmul(
                        q_ps[:, B_LOC * ac : B_LOC * (ac + 1)],
                        WqT_sb[:, qc, 128 * ac : 128 * (ac + 1)],
                        qT_sb[:, qc, :],
                        start=(qc == 0),
                        stop=(qc == 1),
                    )
            # qbias = q + bq + bv  (tanh bias, per-partition)
            qbias = const_pool.tile([128, 2 * B_LOC], F32)
            for ac in range(2):
                nc.vector.tensor_scalar(
                    qbias[:, B_LOC * ac : B_LOC * (ac + 1)],
                    q_ps[:, B_LOC * ac : B_LOC * (ac + 1)],
                    bq_sb[:, ac : ac + 1],
                    bv_sb[:, ac : ac + 1],
                    op0=mybir.AluOpType.add,
                    op1=mybir.AluOpType.add,
                )

            # e accumulator: row 32*b + s holds e[b, 512*s : 512*(s+1)]
            e_ps = ps_e.tile([128, 512], F32, tag="e")
            # c accumulator + per-batch sums: cols 0..255 = c, col 256 = sum
            cs_ps = ps_c.tile([128, D + 8], F32, tag="csums")

            n_emm = 0

            # ---------------- main per-batch pipeline ----------------
            def score(r, b):
                if (r, b) not in v_nats:
                    prefetch(r, b)
                v_nat = v_nats[(r, b)]
                nonlocal n_emm
                for g in range(4):  # t groups of 1024
                    vT_sbs = []
                    for vc in range(2):
                        vT_ps = ps_vt.tile([128, 1024], BF16, tag="vt")
                        for cc in range(8):
                            nc.tensor.transpose(
                                vT_ps[:, 128 * cc : 128 * (cc + 1)],
                                v_nat[:, 8 * g + cc, 128 * vc : 128 * (vc + 1)],
                                id_bf[:, :],
                            )
                        vT_sb = vtsb_pool.tile([128, 1024], BF16, tag="vtsb")
                        nc.vector.tensor_copy(vT_sb[:, :], vT_ps[:, :])
                        vT_sbs.append(vT_sb)

                    for ac in range(2):
                        if skip_keys:
                            break
                        k_ps = ps_keys.tile([128, 1024], F32, tag="keys")
                        for vc in range(2):
                            for h in range(2):
                                nc.tensor.matmul(
                                    k_ps[:, 512 * h : 512 * (h + 1)],
                                    WvT_sb[:, vc, 128 * ac : 128 * (ac + 1)],
                                    vT_sbs[vc][:, 512 * h : 512 * (h + 1)],
                                    start=(vc == 0),
                                    stop=(vc == 1),
                                )
                        tanhT = tanh_pool.tile([128, 1024], BF16, tag="tanh")
                        nc.scalar.activation(
                            tanhT[:, :],
                            k_ps[:, :],
                            AF.Tanh,
                            bias=qbias[:, B_LOC * ac + b : B_LOC * ac + b + 1],
                        )
                        for h in range(2):
                            if skip_e:
                                break
                            s = 2 * g + h
                            nc.tensor.matmul(
                                e_ps[32 * b : 32 * b + 32, :],
                                W_band[:, ac, 128 - s : 160 - s],
                                tanhT[:, 512 * h : 512 * (h + 1)],
                                start=(n_emm % 16 == 0),
                                stop=(n_emm % 16 == 15),
                                skip_group_check=True,
                                tile_position=(0, 32 * b),
                            )
                            n_emm += 1

            def tail(r, b):
                # softmax (no max subtraction; |e| is small) + denominator
                r0 = 32 * b
                if skip_keys or skip_e:
                    return
                v_nat = v_nats[(r, b)]
                nc.scalar.activation(
                    a_unnorm[r0 : r0 + 8, :],
                    e_ps[r0 : r0 + 8, :],
                    AF.Exp,
                    accum_out=accum_sb[r0 : r0 + 8, :],
                )
                # replicate sum of the 8 slice-rows to all 8 rows: ones8.T @ accum
                nc.tensor.matmul(
                    cs_ps[r0 : r0 + 8, D : D + 1],
                    ones8[r0 : r0 + 8, :],
                    accum_sb[r0 : r0 + 8, :],
                    start=True,
                    stop=True,
                    skip_group_check=True,
                    tile_position=(r0, r0),
                )
                nc.vector.reciprocal(
                    recip_sb[r0 : r0 + 8, :], cs_ps[r0 : r0 + 8, D : D + 1]
                )
                nc.vector.tensor_scalar_mul(
                    a_norm[r0 : r0 + 8, :],
                    a_unnorm[r0 : r0 + 8, :],
                    recip_sb[r0 : r0 + 8, 0:1],
                )
                # stream normalized attention out (bf16 -> f32 cast DMA)
                nc.gpsimd.dma_start(
                    a_out[b].rearrange("(s u) -> s u", s=8), a_norm[r0 : r0 + 8, :]
                )

                # transpose a_norm into [t_lo, 32*cb + s] columns for the c matmul
                if skip_c:
                    return
                aT_ps = ps_vt.tile([128, 1024], BF16, tag="vt")
                for cb in range(4):
                    nc.tensor.transpose(
                        aT_ps[:, 32 * cb : 32 * (cb + 1)],
                        a_norm[r0 : r0 + 32, 128 * cb : 128 * (cb + 1)],
                        id_bf[r0 : r0 + 32, r0 : r0 + 32],
                        tile_position=(r0, 0),
                    )
                aT_sb = atsb_pool.tile([128, 128], BF16, tag="atsb")
                nc.vector.tensor_copy(aT_sb[:, :], aT_ps[:, 0:128])

                # c[b] = a_norm[b] @ values[b]: 4 concurrent 32-col PE tiles
                # (round-robin so they run concurrently); partial j lands in
                # PSUM row 32j.
                mm_order = (
                    [(m, j) for m in range(8) for j in range(4)]
                    if ct_c
                    else [(m, j) for j in range(4) for m in range(8)]
                )
                for m, j in mm_order:
                    s = 2 * j + m // 4
                    cb = m % 4
                    chunk = 4 * s + cb
                    nc.tensor.matmul(
                        cs_ps[32 * j : 32 * j + 1, 0:D],
                        aT_sb[:, 32 * cb + s : 32 * cb + s + 1],
                        v_nat[:, chunk, :],
                        start=(m == 0),
                        stop=(m == 7),
                        skip_group_check=True,
                        tile_position=(0, 32 * j),
                    )
                for j in range(4):
                    dst = c_sb[32 * j : 32 * j + 1, D * b : D * (b + 1)]
                    srcp = cs_ps[32 * j : 32 * j + 1, 0:D]
                    if j % 2 == 0:
                        nc.vector.tensor_copy(dst, srcp)
                    else:
                        nc.scalar.activation(dst, srcp, AF.Copy)
                # combine the 4 partial rows: PE-transpose the [128, 256]
                # block (partials are rows {0,32,64,96}) and reduce the
                # strided columns on DVE -> c^T[v] columns in cT_sb.
                cT_ps = ps_vt.tile([128, 256], F32, tag="vt")
                for vc in range(2):
                    nc.tensor.transpose(
                        cT_ps[:, 128 * vc : 128 * (vc + 1)],
                        c_sb[:, D * b + 128 * vc : D * b + 128 * (vc + 1)],
                        id_f32[:, :],
                    )
                for vc in range(2):
                    nc.vector.reduce_sum(
                        cT_sb[:, b, vc : vc + 1],
                        cT_ps[:, 128 * vc : 128 * vc + 97 : 32],
                        axis=mybir.AxisListType.X,
                    )

            # software pipeline: defer each batch's tail until later
            # batches' scoring work is issued, so tail ops fill bubbles.
            # (reps > 1 repeats the whole pipeline for steady-state timing)
            stages = [(r, b) for r in range(reps) for b in range(B_LOC)]
            for i, (r, b) in enumerate(stages):
                score(r, b)
                if i >= tail_lag:
                    tail(*stages[i - tail_lag])
            for r, b in stages[len(stages) - tail_lag :]:
                tail(r, b)

            nc.sync.dma_start(
                c_out.rearrange("b (vc p) -> p b vc", p=128), cT_sb[:, :, :]
            )

    nc.compile()
    if cache:
        _NC_CACHE = nc
    return nc


def make_in_maps(query, values, Wq, bq, Wv, bv, wv_score, **_ignored):
    def f32(x):
        return np.ascontiguousarray(np.asarray(x, dtype=np.float32))

    maps = []
    for i in range(N_CORES):
        sl = slice(B_LOC * i, B_LOC * (i + 1))
        maps.append(
            {
                "values": f32(values[sl]),
                "query": f32(query[sl]),
                "Wq": f32(Wq),
                "bq": f32(bq),
                "Wv": f32(Wv),
                "bv": f32(bv),
                "wv_score": f32(wv_score),
            }
        )
    return maps


def kernel(**inputs):
    nc = build_nc()
    in_maps = make_in_maps(**inputs)
    res = run_bass_kernel_spmd(nc, in_maps, core_ids=list(range(N_CORES)))
    c = np.concatenate([res.results[i]["c_out"] for i in range(N_CORES)], axis=0)
    a = np.concatenate([res.results[i]["a_out"] for i in range(N_CORES)], axis=0)
    return (c, a)
